# revision 28
# baseline (speedup 1.0000x reference)
"""Trainium2 Bass kernel for nn_EngramModule: single-query top-k memory attention
with gated residual + LayerNorm, data-parallel across 8 NeuronCores.

Contract: kernel(**inputs) takes the FULL unsharded inputs and returns the FULL
(8192, 1024) float32 output.

Per-core pipeline (1024 batch rows, 8 row-tiles of 128):
  A+B (fused): per tile, Q = h @ Wq (bf16) then per k-slot: Kp projection in
      fp8e4 DoubleRow (2 contraction chunks per instruction, 2x bf16 rate;
      Wk host-scaled by 32, folded into the exp scale); Vp projection in
      bf16; scores = per-head reduce of q*Kp (DVE); e = exp(scores*scale)
      per-k on ScalarE; weighted V: mult on DVE, running add on GpSimd.
  C:  software-pipelined by one tile on the PE
      (aoT(t) | gate2(t-1) | Wo(t) | gate1(t) | moT(t)) so the ScalarE
      PSUM->SBUF casts always have a PE block in front of their consumer.
      memory_out = attnout @ Wo (bf16); gate = [h|mo] @ Wg in fp8e4
      DoubleRow (Wg host-scaled by 32, mo-half by another 2 since the
      kernel feeds 0.5*mo; sigmoid(x) = 0.5*tanh(x/2)+0.5); aug = h+g*mo;
      LayerNorm per tile: sums via DVE accumulators, Newton rsqrt.

fp8 is used only where the quantization error budget allows (K-side + gate,
~0.015 rel err vs the 2e-2 gate); V/Q/Wo stay bf16. Bulk weights and phase C
inputs ride the ScalarE HWDGE queue; per-(tile,k) activation streams and
outputs ride the SyncE queue. Activations are pre-laid-out on host (pure
transpose/reshape + dtype casts) so contraction dims sit on SBUF partitions.
"""

import os
import sys

import numpy as np

for _p in ("/opt/trn_rl_repo", "/root/.axon_site/_ro/trn_rl_repo"):
    if os.path.isdir(_p) and _p not in sys.path:
        sys.path.insert(0, _p)

from contextlib import ExitStack

import concourse.bacc as bacc
import concourse.mybir as mybir
import concourse.tile as tile
from concourse.bass_utils import run_bass_kernel_spmd

F32 = mybir.dt.float32
BF16 = mybir.dt.bfloat16
F8 = mybir.dt.float8e4
I32 = mybir.dt.int32
AX = mybir.AxisListType
OP = mybir.AluOpType
AF = mybir.ActivationFunctionType
DRM = mybir.MatmulPerfMode.DoubleRow

N_CORES = 8
B = 8192
HID = 1024
NH = 16
DH = 64
TOPK = 8
LN_EPS = 1e-5

BC = B // N_CORES          # rows per core = 1024
NT = BC // 128             # row-tiles per core = 8
NIC = HID // 128           # 128-row contraction chunks = 8
NJH = HID // 512           # 512-wide output halves = 2
WSC = 32.0                 # host scale on fp8 weights
SCALE = DH ** -0.5
RSQRT_MAGIC = 0x5F3759DF

# Set by test.py to collect a profile; grading path leaves this off.
TRACE = False

_CACHE = {}


def _build(nt=NT):
    nc = bacc.Bacc("TRN2", target_bir_lowering=False, debug=False,
                   num_devices=N_CORES)

    # ---- DRAM parameters (per-core shard, host-prepped layouts) ----
    h_d = nc.declare_dram_parameter("h", [nt, 128, HID], BF16, isOutput=False)
    hTb_d = nc.declare_dram_parameter("hTb", [nt, 128, NIC, 128], BF16, isOutput=False)
    hT8_d = nc.declare_dram_parameter("hT8", [nt, 128, NIC, 128], F8, isOutput=False)
    mkT8_d = nc.declare_dram_parameter("mkT8", [nt, TOPK, 128, NIC, 128], F8, isOutput=False)
    mvT_d = nc.declare_dram_parameter("mvT", [nt, TOPK, 128, NIC, 128], BF16, isOutput=False)
    wq_d = nc.declare_dram_parameter("Wq", [128, NIC, HID], BF16, isOutput=False)
    wk8_d = nc.declare_dram_parameter("Wk8", [128, NIC, HID], F8, isOutput=False)
    wv_d = nc.declare_dram_parameter("Wv", [128, NIC, HID], BF16, isOutput=False)
    wo_d = nc.declare_dram_parameter("Wo", [128, NIC, HID], BF16, isOutput=False)
    wg8_d = nc.declare_dram_parameter("Wg8", [128, NIC, HID], F8, isOutput=False)
    wom8_d = nc.declare_dram_parameter("Wom8", [128, NIC, HID], F8, isOutput=False)
    bgb_d = nc.declare_dram_parameter("bgB", [128, HID], BF16, isOutput=False)
    eyeb_d = nc.declare_dram_parameter("eyeb", [128, 128], BF16, isOutput=False)
    lng_d = nc.declare_dram_parameter("lngB", [128, HID], BF16, isOutput=False)
    lnb_d = nc.declare_dram_parameter("lnbB", [128, HID], BF16, isOutput=False)
    out_d = nc.declare_dram_parameter("out", [nt, 128, HID], F32, isOutput=True)

    def load_w(tile_sb, dram, nchunk):
        # bulk weights on the ScalarE HWDGE queue, chunked so the first
        # dependent matmul only waits for its own chunk
        for ic in range(nchunk):
            nc.scalar.dma_start(tile_sb[:, ic, :], dram.ap()[:, ic, :])

    with ExitStack() as octx:
        tc = octx.enter_context(tile.TileContext(nc))

        pers = octx.enter_context(tc.tile_pool(name="pers", bufs=1))
        sum_all = pers.tile([128, nt], F32, tag="sum_all")
        ss_all = pers.tile([128, nt], F32, tag="ss_all")
        # phase C constants, DMAed during the A+B head so C never waits
        eyeb_sb = pers.tile([128, 128], BF16, tag="eyeb")
        bgb_sb = pers.tile([128, HID], BF16, tag="bgb")
        lng_sb = pers.tile([128, HID], BF16, tag="lng")
        lnb_sb = pers.tile([128, HID], BF16, tag="lnb")

        # Wo/Wg are loaded during A+B (scalar queue) and consumed in C
        pWO_cm = tc.tile_pool(name="pWO", bufs=1); pWO = pWO_cm.__enter__()
        wo_sb = pWO.tile([128, NIC, HID], BF16, tag="wo")
        wg8_sb = pWO.tile([128, NIC, HID], F8, tag="wg8")
        wom8_sb = pWO.tile([128, NIC, HID], F8, tag="wom8")

        # attnout stays SBUF-resident from B into C (bf16, feeds transposes)
        pAO_cm = tc.tile_pool(name="pAO", bufs=1); pAO = pAO_cm.__enter__()
        ao_all = pAO.tile([128, nt, HID], BF16, tag="ao_all")

        # phase C per-tile input stream (created here so B can prefetch t=0)
        cstr_cm = tc.tile_pool(name="c_str", bufs=3); cstr = cstr_cm.__enter__()

        # ========== phase A+B: Q projection fused into attention ==========
        with ExitStack() as bctx:
            wqp = bctx.enter_context(tc.tile_pool(name="wq", bufs=1))
            hp = bctx.enter_context(tc.tile_pool(name="hT_a", bufs=3))
            qp = bctx.enter_context(tc.tile_pool(name="qq", bufs=2))
            wkv = bctx.enter_context(tc.tile_pool(name="wkv", bufs=1))
            mp = bctx.enter_context(tc.tile_pool(name="mkv", bufs=4))
            kvps = bctx.enter_context(tc.tile_pool(name="kv_ps", bufs=2, space="PSUM"))
            sp = bctx.enter_context(tc.tile_pool(name="scr", bufs=2))
            accp = bctx.enter_context(tc.tile_pool(name="acc", bufs=2))
            ep = bctx.enter_context(tc.tile_pool(name="e", bufs=2))

            wq_sb = wqp.tile([128, NIC, HID], BF16, tag="wq")
            wk8_sb = wkv.tile([128, NIC, HID], F8, tag="wk8")
            wv_sb = wkv.tile([128, NIC, HID], BF16, tag="wv")
            load_w(wq_sb, wq_d, NIC)
            load_w(wk8_sb, wk8_d, NIC)
            load_w(wv_sb, wv_d, NIC)
            hT_pre = {}
            for pt in (0, 1):
                ht = hp.tile([128, NIC, 128], BF16, tag="hT")
                nc.sync.dma_start(ht[:], hTb_d.ap()[pt])
                hT_pre[pt] = ht
            preload = {}
            for (pt, pk) in ((0, 0),):
                a = mp.tile([128, NIC, 128], F8, tag="mkT8")
                nc.sync.dma_start(a[:], mkT8_d.ap()[pt, pk])
                b_ = mp.tile([128, NIC, 128], BF16, tag="mvT")
                nc.sync.dma_start(b_[:], mvT_d.ap()[pt, pk])
                preload[(pt, pk)] = (a, b_)

            for t in range(nt):
                # phase C weights/constants trickle in on the sync queue
                # (its own sequencer, no ScalarE cost) spread across tiles
                if t == 1:
                    for ic in range(NIC):
                        nc.sync.dma_start(wo_sb[:, ic, :], wo_d.ap()[:, ic, :])
                elif t == 2:
                    for ic in range(NIC):
                        nc.sync.dma_start(wg8_sb[:, ic, :], wg8_d.ap()[:, ic, :])
                elif t == 3:
                    for ic in range(NIC):
                        nc.sync.dma_start(wom8_sb[:, ic, :], wom8_d.ap()[:, ic, :])
                    nc.sync.dma_start(eyeb_sb[:], eyeb_d.ap())
                    nc.sync.dma_start(bgb_sb[:], bgb_d.ap())
                    nc.sync.dma_start(lng_sb[:], lng_d.ap())
                    nc.sync.dma_start(lnb_sb[:], lnb_d.ap())
                if t in hT_pre:
                    hT_t = hT_pre[t]
                else:
                    hT_t = hp.tile([128, NIC, 128], BF16, tag="hT")
                    nc.sync.dma_start(hT_t[:], hTb_d.ap()[t])

                # Q projection for this tile (PSUM slot shared with kp)
                q_ps = kvps.tile([128, HID], F32, tag="kp")
                for ic in range(NIC):
                    for jh in range(NJH):
                        nc.tensor.matmul(
                            q_ps[:, jh * 512:(jh + 1) * 512],
                            hT_t[:, ic, :],
                            wq_sb[:, ic, jh * 512:(jh + 1) * 512],
                            start=(ic == 0), stop=(ic == NIC - 1),
                        )
                q_t = qp.tile([128, HID], BF16, tag="q")
                nc.scalar.copy(q_t[:], q_ps[:])

                acc = accp.tile([128, HID], F32, tag="acc")
                e_all = ep.tile([128, TOPK, NH], F32, tag="e_all")
                for k in range(TOPK):
                    if (t, k) in preload:
                        mkT8, mvT = preload[(t, k)]
                    else:
                        mkT8 = mp.tile([128, NIC, 128], F8, tag="mkT8")
                        nc.sync.dma_start(mkT8[:], mkT8_d.ap()[t, k])
                        mvT = mp.tile([128, NIC, 128], BF16, tag="mvT")
                        nc.sync.dma_start(mvT[:], mvT_d.ap()[t, k])

                    # Kp in fp8 DoubleRow: 2 contraction chunks per matmul
                    kp_ps = kvps.tile([128, HID], F32, tag="kp")
                    for icp in range(NIC // 2):
                        for jh in range(NJH):
                            nc.tensor.matmul(
                                kp_ps[:, jh * 512:(jh + 1) * 512],
                                mkT8[:, 2 * icp:2 * icp + 2, :],
                                wk8_sb[:, 2 * icp:2 * icp + 2, jh * 512:(jh + 1) * 512],
                                start=(icp == 0), stop=(icp == NIC // 2 - 1),
                                perf_mode=DRM,
                            )
                    # Vp in bf16
                    vp_ps = kvps.tile([128, HID], F32, tag="vp")
                    for ic in range(NIC):
                        for jh in range(NJH):
                            nc.tensor.matmul(
                                vp_ps[:, jh * 512:(jh + 1) * 512],
                                mvT[:, ic, :],
                                wv_sb[:, ic, jh * 512:(jh + 1) * 512],
                                start=(ic == 0), stop=(ic == NIC - 1),
                            )

                    # scores for all 16 heads of this k-slot
                    p_scr = sp.tile([128, HID], BF16, tag="p")
                    nc.vector.tensor_mul(p_scr[:], q_t[:], kp_ps[:])
                    s_k = ep.tile([128, NH], F32, tag="s_k")
                    nc.vector.reduce_sum(
                        s_k[:], p_scr[:].rearrange("p (h d) -> p h d", h=NH), axis=AX.X)
                    # e = exp(scores * DH**-0.5 / WSC); logits ~N(0,1), no max-sub
                    nc.scalar.activation(e_all[:, k, :], s_k[:], AF.Exp,
                                         scale=SCALE / WSC)

                    # weighted V accumulate: DVE mult, GpSimd running add
                    e_bc = e_all[:, k, :].unsqueeze(2).broadcast_to([128, NH, DH])
                    dst = acc if k == 0 else sp.tile([128, HID], F32, tag="pv")
                    nc.vector.tensor_tensor(
                        dst[:].rearrange("p (h d) -> p h d", h=NH),
                        vp_ps[:].rearrange("p (h d) -> p h d", h=NH),
                        e_bc, op=OP.mult)
                    if k > 0:
                        nc.gpsimd.tensor_add(acc[:], acc[:], dst[:])

                # normalize: attnout = acc * (1/sum_k e), written bf16
                den = ep.tile([128, NH], F32, tag="den")
                nc.vector.reduce_sum(
                    den[:], e_all[:].rearrange("p k h -> p h k"), axis=AX.X)
                rden = ep.tile([128, NH], F32, tag="rden")
                nc.vector.reciprocal(rden[:], den[:])
                rden_bc = rden[:].unsqueeze(2).broadcast_to([128, NH, DH])
                with nc.allow_low_precision(reason="attnout bf16 feeds bf16 matmul"):
                    nc.vector.tensor_tensor(
                        ao_all[:, t, :].rearrange("p (h d) -> p h d", h=NH),
                        acc[:].rearrange("p (h d) -> p h d", h=NH),
                        rden_bc, op=OP.mult)

            # prefetch phase C tile-0 inputs (scalar queue is idle by now)
            c_pre = {}
            for pt in (0,):
                a = cstr.tile([128, NIC, 128], F8, tag="hT8_c")
                nc.scalar.dma_start(a[:], hT8_d.ap()[pt])
                b_ = cstr.tile([128, HID], BF16, tag="h_c")
                nc.scalar.dma_start(b_[:], h_d.ap()[pt])
                c_pre[pt] = (a, b_)

        # ===== phase C: memory_out, gate, residual, LayerNorm, output =====
        # gate = h @ Wg_h + attnout @ Wom with Wom = Wo @ Wg_mo folded on the
        # host, so the gate needs only the transposed attnout (atT8) and never
        # waits on memory_out. One-tile software pipeline covers the ScalarE
        # casts: aoT(t) | gate(t-1) | Wo(t).
        with ExitStack() as cctx:
            csb = cctx.enter_context(tc.tile_pool(name="c_sb", bufs=2))
            stp = cctx.enter_context(tc.tile_pool(name="stats", bufs=2))
            tps = cctx.enter_context(tc.tile_pool(name="tp_ps", bufs=2, space="PSUM"))
            mps = cctx.enter_context(tc.tile_pool(name="mo_ps", bufs=1, space="PSUM"))
            gps = cctx.enter_context(tc.tile_pool(name="g_ps", bufs=2, space="PSUM"))

            def emit_gate(hT8_sb, atT8_sb):
                g_ps = gps.tile([128, HID], F32, tag="g_ps")
                for icp in range(NIC // 2):
                    for jh in range(NJH):
                        sl = slice(jh * 512, (jh + 1) * 512)
                        nc.tensor.matmul(
                            g_ps[:, sl], hT8_sb[:, 2 * icp:2 * icp + 2, :],
                            wg8_sb[:, 2 * icp:2 * icp + 2, sl],
                            start=(icp == 0), stop=False, perf_mode=DRM)
                for icp in range(NIC // 2):
                    for jh in range(NJH):
                        sl = slice(jh * 512, (jh + 1) * 512)
                        nc.tensor.matmul(
                            g_ps[:, sl], atT8_sb[:, 2 * icp:2 * icp + 2, :],
                            wom8_sb[:, 2 * icp:2 * icp + 2, sl],
                            start=False, stop=(icp == NIC // 2 - 1),
                            perf_mode=DRM)
                return g_ps

            def epilogue(t, g_ps, h_sb, mob_sb, last=False):
                ew = nc.vector if last else nc.gpsimd
                gb_sb = csb.tile([128, HID], F32, tag="gb")
                nc.vector.tensor_add(gb_sb[:], g_ps[:], bgb_sb[:])
                # sigmoid(x) = 0.5*tanh(x/2) + 0.5; 1/WSC descales Wg8/Wom8
                nc.scalar.activation(gb_sb[:], gb_sb[:], AF.Tanh,
                                     scale=0.5 / WSC)
                # aug = h + g*mo = (h + mob) + tanh*mob with mob = 0.5*mo
                u_sb = csb.tile([128, HID], F32, tag="u")
                ew.tensor_add(u_sb[:], h_sb[:], mob_sb[:])
                v_sb = csb.tile([128, HID], F32, tag="v")
                ew.tensor_mul(v_sb[:], gb_sb[:], mob_sb[:])
                nc.vector.scalar_tensor_tensor(
                    u_sb[:], u_sb[:], 0.0, v_sb[:], op0=OP.add, op1=OP.add,
                    accum_out=sum_all[:, t:t + 1])
                # sum of squares: (u+0)*u on DVE, keeping only the accumulator
                nc.vector.scalar_tensor_tensor(
                    v_sb[:], u_sb[:], 0.0, u_sb[:], op0=OP.add, op1=OP.mult,
                    accum_out=ss_all[:, t:t + 1])

                # ---- LayerNorm finalize, per tile, VectorE only ----
                mean = stp.tile([128, 1], F32, tag="mean")
                nc.vector.tensor_scalar_mul(mean[:], sum_all[:, t:t + 1], 1.0 / HID)
                m2 = stp.tile([128, 1], F32, tag="m2")
                nc.vector.tensor_mul(m2[:], mean[:], mean[:])
                nc.vector.tensor_scalar_add(m2[:], m2[:], -LN_EPS)
                vpe = stp.tile([128, 1], F32, tag="vpe")
                nc.vector.scalar_tensor_tensor(
                    vpe[:], ss_all[:, t:t + 1], 1.0 / HID, m2[:],
                    op0=OP.mult, op1=OP.subtract)
                # rstd = 1/sqrt(vpe): quake init + 2 Newton iterations
                y = stp.tile([128, 1], F32, tag="y")
                yi = y[:].bitcast(I32)
                nc.vector.tensor_scalar(
                    yi, vpe[:].bitcast(I32), 1, None,
                    op0=OP.logical_shift_right)
                nc.vector.tensor_scalar(
                    yi, yi, -RSQRT_MAGIC, -1,
                    op0=OP.add, op1=OP.mult)
                yy = stp.tile([128, 1], F32, tag="yy")
                hw = stp.tile([128, 1], F32, tag="hw")
                for _ in range(2):
                    nc.vector.tensor_mul(yy[:], y[:], y[:])
                    nc.vector.tensor_mul(yy[:], yy[:], vpe[:])
                    nc.vector.tensor_scalar(
                        hw[:], yy[:], -0.5, 1.5, op0=OP.mult, op1=OP.add)
                    nc.vector.tensor_mul(y[:], y[:], hw[:])

                # yout = (aug - mean)*rstd*lng + lnb
                nc.vector.scalar_tensor_tensor(
                    u_sb[:], u_sb[:], mean[:], lng_sb[:],
                    op0=OP.subtract, op1=OP.mult)
                yo_sb = csb.tile([128, HID], F32, tag="yo")
                nc.vector.scalar_tensor_tensor(
                    yo_sb[:], u_sb[:], y[:], lnb_sb[:],
                    op0=OP.mult, op1=OP.add)
                nc.sync.dma_start(out_d.ap()[t], yo_sb[:])

            prev = None  # (t, hT8, atT8, h, mob)
            pend = None  # (t, g_ps, h, mob) awaiting epilogue
            for t in range(nt):
                if t in c_pre:
                    hT8_sb, h_sb = c_pre[t]
                else:
                    hT8_sb = cstr.tile([128, NIC, 128], F8, tag="hT8_c")
                    nc.scalar.dma_start(hT8_sb[:], hT8_d.ap()[t])
                    h_sb = cstr.tile([128, HID], BF16, tag="h_c")
                    nc.scalar.dma_start(h_sb[:], h_d.ap()[t])

                # attnout^T via bf16 PE transposes; cast to bf16 (Wo) + f8 (gate)
                at_ps = tps.tile([128, NIC, 128], BF16, tag="tp_ps")
                for ic in range(NIC):
                    nc.tensor.transpose(
                        at_ps[:, ic, :], ao_all[:, t, ic * 128:(ic + 1) * 128],
                        eyeb_sb[:])
                atT_sb = csb.tile([128, NIC, 128], BF16, tag="atT")
                nc.scalar.copy(atT_sb[:], at_ps[:])
                atT8_sb = csb.tile([128, NIC, 128], F8, tag="atT8")
                nc.scalar.copy(atT8_sb[:], at_ps[:])

                # previous tile's gate fills the PE while atT copies out
                if prev is not None:
                    pt, phT8, patT8, ph, pmob = prev
                    pend = (pt, emit_gate(phT8, patT8), ph, pmob)

                mo_ps = mps.tile([128, HID], F32, tag="mo_ps")
                for ic in range(NIC):
                    for jh in range(NJH):
                        nc.tensor.matmul(
                            mo_ps[:, jh * 512:(jh + 1) * 512],
                            atT_sb[:, ic, :],
                            wo_sb[:, ic, jh * 512:(jh + 1) * 512],
                            start=(ic == 0), stop=(ic == NIC - 1),
                        )
                # mob holds 0.5*mo (bf16), used only by the aug math
                mob_sb = csb.tile([128, HID], BF16, tag="mob")
                nc.scalar.activation(mob_sb[:], mo_ps[:], AF.Copy, scale=0.5)

                if pend is not None:
                    epilogue(*pend)
                    pend = None
                prev = (t, hT8_sb, atT8_sb, h_sb, mob_sb)

            pt, phT8, patT8, ph, pmob = prev
            g_last = emit_gate(phT8, patT8)
            epilogue(pt, g_last, ph, pmob, last=True)

        cstr_cm.__exit__(None, None, None)
        pAO_cm.__exit__(None, None, None)   # release attnout
        pWO_cm.__exit__(None, None, None)   # release Wo/Wg

    nc.compile()
    return nc


def _prep_core(hs, mk, mv, nt, bf, e4):
    """Host-side layout prep for one core's shard (transpose/reshape + casts)."""
    hT = np.ascontiguousarray(
        hs.reshape(nt, 128, NIC, 128).transpose(0, 3, 2, 1))      # [t,p,ic,b]
    h = np.ascontiguousarray(hs.reshape(nt, 128, HID)).astype(bf)
    mkT = np.ascontiguousarray(
        mk.reshape(nt, 128, TOPK, NIC, 128).transpose(0, 2, 4, 3, 1))
    mvT = np.ascontiguousarray(
        mv.reshape(nt, 128, TOPK, NIC, 128).transpose(0, 2, 4, 3, 1))
    return hT.astype(bf), hT.astype(e4), h, mkT.astype(e4), mvT.astype(bf)


def kernel(**inputs):
    hs = np.asarray(inputs["hidden_state"], dtype=np.float32)
    mk = np.asarray(inputs["memory_keys"], dtype=np.float32)
    mv = np.asarray(inputs["memory_values"], dtype=np.float32)

    import ml_dtypes
    bf = ml_dtypes.bfloat16
    e4 = ml_dtypes.float8_e4m3
    wq = np.ascontiguousarray(
        np.asarray(inputs["Wq"], np.float32).reshape(NIC, 128, HID).transpose(1, 0, 2)).astype(bf)
    wk8 = np.ascontiguousarray(
        (np.asarray(inputs["Wk"], np.float32) * WSC).reshape(NIC, 128, HID).transpose(1, 0, 2)).astype(e4)
    wv = np.ascontiguousarray(
        np.asarray(inputs["Wv"], np.float32).reshape(NIC, 128, HID).transpose(1, 0, 2)).astype(bf)
    wo = np.ascontiguousarray(
        np.asarray(inputs["Wo"], np.float32).reshape(NIC, 128, HID).transpose(1, 0, 2)).astype(bf)
    wg_f = np.asarray(inputs["Wg"], np.float32)
    wg8 = np.ascontiguousarray(
        (wg_f[:HID] * WSC).reshape(NIC, 128, HID).transpose(1, 0, 2)).astype(e4)
    # gate mo-half folded through Wo on the host: ao @ (Wo @ Wg_mo)
    wom = (np.asarray(inputs["Wo"], np.float32) @ wg_f[HID:]) * WSC
    wom8 = np.ascontiguousarray(
        wom.reshape(NIC, 128, HID).transpose(1, 0, 2)).astype(e4)
    bgb = np.ascontiguousarray(
        np.broadcast_to(np.asarray(inputs["bg"], np.float32), (128, HID))).astype(bf)
    lng = np.ascontiguousarray(
        np.broadcast_to(np.asarray(inputs["ln_g"], np.float32), (128, HID))).astype(bf)
    lnb = np.ascontiguousarray(
        np.broadcast_to(np.asarray(inputs["ln_b"], np.float32), (128, HID))).astype(bf)
    eyeb = np.eye(128, dtype=np.float32).astype(bf)

    if "nc" not in _CACHE:
        _CACHE["nc"] = _build(NT)
    nc = _CACHE["nc"]

    in_maps = []
    for c in range(N_CORES):
        sl = slice(c * BC, (c + 1) * BC)
        hTb, hT8, h, mkT8, mvT = _prep_core(hs[sl], mk[sl], mv[sl], NT, bf, e4)
        in_maps.append({
            "hTb": hTb, "hT8": hT8, "h": h,
            "mkT8": mkT8, "mvT": mvT,
            "Wq": wq, "Wk8": wk8, "Wv": wv, "Wo": wo, "Wg8": wg8,
            "Wom8": wom8,
            "bgB": bgb, "eyeb": eyeb, "lngB": lng, "lnbB": lnb,
        })

    res = run_bass_kernel_spmd(nc, in_maps, core_ids=list(range(N_CORES)),
                               trace=TRACE)
    kernel.last_result = res
    out = np.concatenate(
        [r["out"].reshape(BC, HID) for r in res.results], axis=0)
    return out


kernel.last_result = None


# revision 29
# speedup vs baseline: 1.1698x; 1.1698x over previous
"""Trainium2 Bass kernel for nn_EngramModule: single-query top-k memory attention
with gated residual + LayerNorm, data-parallel across 8 NeuronCores.

Contract: kernel(**inputs) takes the FULL unsharded inputs and returns the FULL
(8192, 1024) float32 output.

Per-core pipeline (1024 batch rows, 8 row-tiles of 128):
  A+B (fused): per tile, Q = h @ Wq (bf16) then per k-slot: Kp projection in
      fp8e4 DoubleRow (2 contraction chunks per instruction, 2x bf16 rate;
      Wk host-scaled by 32, folded into the exp scale); Vp projection in
      bf16; scores = per-head reduce of q*Kp (DVE); e = exp(scores*scale)
      per-k on ScalarE; weighted V: mult on DVE, running add on GpSimd.
  C:  software-pipelined by one tile on the PE
      (aoT(t) | gate2(t-1) | Wo(t) | gate1(t) | moT(t)) so the ScalarE
      PSUM->SBUF casts always have a PE block in front of their consumer.
      memory_out = attnout @ Wo (bf16); gate = [h|mo] @ Wg in fp8e4
      DoubleRow (Wg host-scaled by 32, mo-half by another 2 since the
      kernel feeds 0.5*mo; sigmoid(x) = 0.5*tanh(x/2)+0.5); aug = h+g*mo;
      LayerNorm per tile: sums via DVE accumulators, Newton rsqrt.

fp8 is used only where the quantization error budget allows (K-side + gate,
~0.015 rel err vs the 2e-2 gate); V/Q/Wo stay bf16. Bulk weights and phase C
inputs ride the ScalarE HWDGE queue; per-(tile,k) activation streams and
outputs ride the SyncE queue. Activations are pre-laid-out on host (pure
transpose/reshape + dtype casts) so contraction dims sit on SBUF partitions.
"""

import os
import sys

import numpy as np

for _p in ("/opt/trn_rl_repo", "/root/.axon_site/_ro/trn_rl_repo"):
    if os.path.isdir(_p) and _p not in sys.path:
        sys.path.insert(0, _p)

from contextlib import ExitStack

import concourse.bacc as bacc
import concourse.mybir as mybir
import concourse.tile as tile
from concourse.bass_utils import run_bass_kernel_spmd

F32 = mybir.dt.float32
BF16 = mybir.dt.bfloat16
F8 = mybir.dt.float8e4
I32 = mybir.dt.int32
AX = mybir.AxisListType
OP = mybir.AluOpType
AF = mybir.ActivationFunctionType
DRM = mybir.MatmulPerfMode.DoubleRow

N_CORES = 8
B = 8192
HID = 1024
NH = 16
DH = 64
TOPK = 8
LN_EPS = 1e-5

BC = B // N_CORES          # rows per core = 1024
NT = BC // 128             # row-tiles per core = 8
NIC = HID // 128           # 128-row contraction chunks = 8
NJH = HID // 512           # 512-wide output halves = 2
WSC = 32.0                 # host scale on fp8 weights
SCALE = DH ** -0.5
RSQRT_MAGIC = 0x5F3759DF

# Set by test.py to collect a profile; grading path leaves this off.
TRACE = False

_CACHE = {}


def _build(nt=NT):
    nc = bacc.Bacc("TRN2", target_bir_lowering=False, debug=False,
                   num_devices=N_CORES)

    # ---- DRAM parameters (per-core shard, host-prepped layouts) ----
    h_d = nc.declare_dram_parameter("h", [nt, 128, HID], BF16, isOutput=False)
    hTb_d = nc.declare_dram_parameter("hTb", [nt, 128, NIC, 128], BF16, isOutput=False)
    hT8_d = nc.declare_dram_parameter("hT8", [nt, 128, NIC, 128], F8, isOutput=False)
    mkT8_d = nc.declare_dram_parameter("mkT8", [nt, TOPK, 128, NIC, 128], F8, isOutput=False)
    mvT_d = nc.declare_dram_parameter("mvT", [nt, TOPK, 128, NIC, 128], BF16, isOutput=False)
    wq_d = nc.declare_dram_parameter("Wq", [128, NIC, HID], BF16, isOutput=False)
    wk8_d = nc.declare_dram_parameter("Wk8", [128, NIC, HID], F8, isOutput=False)
    wv_d = nc.declare_dram_parameter("Wv", [128, NIC, HID], BF16, isOutput=False)
    wo_d = nc.declare_dram_parameter("Wo", [128, NIC, HID], BF16, isOutput=False)
    wg8_d = nc.declare_dram_parameter("Wg8", [128, NIC, HID], F8, isOutput=False)
    wom_d = nc.declare_dram_parameter("Wom", [128, NIC, HID], BF16, isOutput=False)
    bgb_d = nc.declare_dram_parameter("bgB", [128, HID], BF16, isOutput=False)
    eyeb_d = nc.declare_dram_parameter("eyeb", [128, 128], BF16, isOutput=False)
    lng_d = nc.declare_dram_parameter("lngB", [128, HID], BF16, isOutput=False)
    lnb_d = nc.declare_dram_parameter("lnbB", [128, HID], BF16, isOutput=False)
    out_d = nc.declare_dram_parameter("out", [nt, 128, HID], F32, isOutput=True)

    def load_w(tile_sb, dram, nchunk):
        # bulk weights on the ScalarE HWDGE queue, chunked so the first
        # dependent matmul only waits for its own chunk
        for ic in range(nchunk):
            nc.scalar.dma_start(tile_sb[:, ic, :], dram.ap()[:, ic, :])

    with ExitStack() as octx:
        tc = octx.enter_context(tile.TileContext(nc))

        pers = octx.enter_context(tc.tile_pool(name="pers", bufs=1))
        sum_all = pers.tile([128, nt], F32, tag="sum_all")
        ss_all = pers.tile([128, nt], F32, tag="ss_all")
        # phase C constants, DMAed during the A+B head so C never waits
        eyeb_sb = pers.tile([128, 128], BF16, tag="eyeb")
        bgb_sb = pers.tile([128, HID], BF16, tag="bgb")
        lng_sb = pers.tile([128, HID], BF16, tag="lng")
        lnb_sb = pers.tile([128, HID], BF16, tag="lnb")

        # Wo/Wg are loaded during A+B (scalar queue) and consumed in C
        pWO_cm = tc.tile_pool(name="pWO", bufs=1); pWO = pWO_cm.__enter__()
        wo_sb = pWO.tile([128, NIC, HID], BF16, tag="wo")
        wg8_sb = pWO.tile([128, NIC, HID], F8, tag="wg8")
        wom_sb = pWO.tile([128, NIC, HID], BF16, tag="wom")

        # attnout stays SBUF-resident from B into C (bf16, feeds transposes)
        pAO_cm = tc.tile_pool(name="pAO", bufs=1); pAO = pAO_cm.__enter__()
        ao_all = pAO.tile([128, nt, HID], BF16, tag="ao_all")

        # phase C per-tile input stream (created here so B can prefetch t=0)
        cstr_cm = tc.tile_pool(name="c_str", bufs=3); cstr = cstr_cm.__enter__()

        # ========== phase A+B: Q projection fused into attention ==========
        with ExitStack() as bctx:
            wqp = bctx.enter_context(tc.tile_pool(name="wq", bufs=1))
            hp = bctx.enter_context(tc.tile_pool(name="hT_a", bufs=3))
            qp = bctx.enter_context(tc.tile_pool(name="qq", bufs=2))
            wkv = bctx.enter_context(tc.tile_pool(name="wkv", bufs=1))
            mp = bctx.enter_context(tc.tile_pool(name="mkv", bufs=4))
            kvps = bctx.enter_context(tc.tile_pool(name="kv_ps", bufs=2, space="PSUM"))
            sp = bctx.enter_context(tc.tile_pool(name="scr", bufs=2))
            accp = bctx.enter_context(tc.tile_pool(name="acc", bufs=2))
            ep = bctx.enter_context(tc.tile_pool(name="e", bufs=2))

            wq_sb = wqp.tile([128, NIC, HID], BF16, tag="wq")
            wk8_sb = wkv.tile([128, NIC, HID], F8, tag="wk8")
            wv_sb = wkv.tile([128, NIC, HID], BF16, tag="wv")
            load_w(wq_sb, wq_d, NIC)
            load_w(wk8_sb, wk8_d, NIC)
            load_w(wv_sb, wv_d, NIC)
            hT_pre = {}
            for pt in (0, 1):
                ht = hp.tile([128, NIC, 128], BF16, tag="hT")
                nc.sync.dma_start(ht[:], hTb_d.ap()[pt])
                hT_pre[pt] = ht
            preload = {}
            for (pt, pk) in ((0, 0),):
                a = mp.tile([128, NIC, 128], F8, tag="mkT8")
                nc.sync.dma_start(a[:], mkT8_d.ap()[pt, pk])
                b_ = mp.tile([128, NIC, 128], BF16, tag="mvT")
                nc.sync.dma_start(b_[:], mvT_d.ap()[pt, pk])
                preload[(pt, pk)] = (a, b_)

            for t in range(nt):
                # phase C weights/constants trickle in on the sync queue
                # (its own sequencer, no ScalarE cost) spread across tiles
                if t == 1:
                    for ic in range(NIC):
                        nc.sync.dma_start(wo_sb[:, ic, :], wo_d.ap()[:, ic, :])
                elif t == 2:
                    for ic in range(NIC):
                        nc.sync.dma_start(wg8_sb[:, ic, :], wg8_d.ap()[:, ic, :])
                elif t == 3:
                    for ic in range(NIC):
                        nc.sync.dma_start(wom_sb[:, ic, :], wom_d.ap()[:, ic, :])
                    nc.sync.dma_start(eyeb_sb[:], eyeb_d.ap())
                    nc.sync.dma_start(bgb_sb[:], bgb_d.ap())
                    nc.sync.dma_start(lng_sb[:], lng_d.ap())
                    nc.sync.dma_start(lnb_sb[:], lnb_d.ap())
                if t in hT_pre:
                    hT_t = hT_pre[t]
                else:
                    hT_t = hp.tile([128, NIC, 128], BF16, tag="hT")
                    nc.sync.dma_start(hT_t[:], hTb_d.ap()[t])

                # Q projection for this tile (PSUM slot shared with kp)
                q_ps = kvps.tile([128, HID], F32, tag="kp")
                for ic in range(NIC):
                    for jh in range(NJH):
                        nc.tensor.matmul(
                            q_ps[:, jh * 512:(jh + 1) * 512],
                            hT_t[:, ic, :],
                            wq_sb[:, ic, jh * 512:(jh + 1) * 512],
                            start=(ic == 0), stop=(ic == NIC - 1),
                        )
                q_t = qp.tile([128, HID], BF16, tag="q")
                nc.scalar.copy(q_t[:], q_ps[:])

                acc = accp.tile([128, HID], F32, tag="acc")
                e_all = ep.tile([128, TOPK, NH], F32, tag="e_all")
                for k in range(TOPK):
                    if (t, k) in preload:
                        mkT8, mvT = preload[(t, k)]
                    else:
                        mkT8 = mp.tile([128, NIC, 128], F8, tag="mkT8")
                        nc.sync.dma_start(mkT8[:], mkT8_d.ap()[t, k])
                        mvT = mp.tile([128, NIC, 128], BF16, tag="mvT")
                        nc.sync.dma_start(mvT[:], mvT_d.ap()[t, k])

                    # Kp in fp8 DoubleRow: 2 contraction chunks per matmul
                    kp_ps = kvps.tile([128, HID], F32, tag="kp")
                    for icp in range(NIC // 2):
                        for jh in range(NJH):
                            nc.tensor.matmul(
                                kp_ps[:, jh * 512:(jh + 1) * 512],
                                mkT8[:, 2 * icp:2 * icp + 2, :],
                                wk8_sb[:, 2 * icp:2 * icp + 2, jh * 512:(jh + 1) * 512],
                                start=(icp == 0), stop=(icp == NIC // 2 - 1),
                                perf_mode=DRM,
                            )
                    # Vp in bf16
                    vp_ps = kvps.tile([128, HID], F32, tag="vp")
                    for ic in range(NIC):
                        for jh in range(NJH):
                            nc.tensor.matmul(
                                vp_ps[:, jh * 512:(jh + 1) * 512],
                                mvT[:, ic, :],
                                wv_sb[:, ic, jh * 512:(jh + 1) * 512],
                                start=(ic == 0), stop=(ic == NIC - 1),
                            )

                    # scores for all 16 heads of this k-slot
                    p_scr = sp.tile([128, HID], BF16, tag="p")
                    nc.vector.tensor_mul(p_scr[:], q_t[:], kp_ps[:])
                    s_k = ep.tile([128, NH], F32, tag="s_k")
                    nc.vector.reduce_sum(
                        s_k[:], p_scr[:].rearrange("p (h d) -> p h d", h=NH), axis=AX.X)
                    # e = exp(scores * DH**-0.5 / WSC); logits ~N(0,1), no max-sub
                    nc.scalar.activation(e_all[:, k, :], s_k[:], AF.Exp,
                                         scale=SCALE / WSC)

                    # weighted V accumulate: DVE mult, GpSimd running add
                    e_bc = e_all[:, k, :].unsqueeze(2).broadcast_to([128, NH, DH])
                    dst = acc if k == 0 else sp.tile([128, HID], F32, tag="pv")
                    nc.vector.tensor_tensor(
                        dst[:].rearrange("p (h d) -> p h d", h=NH),
                        vp_ps[:].rearrange("p (h d) -> p h d", h=NH),
                        e_bc, op=OP.mult)
                    if k > 0:
                        nc.gpsimd.tensor_add(acc[:], acc[:], dst[:])

                # normalize: attnout = acc * (1/sum_k e), written bf16
                den = ep.tile([128, NH], F32, tag="den")
                nc.vector.reduce_sum(
                    den[:], e_all[:].rearrange("p k h -> p h k"), axis=AX.X)
                rden = ep.tile([128, NH], F32, tag="rden")
                nc.vector.reciprocal(rden[:], den[:])
                rden_bc = rden[:].unsqueeze(2).broadcast_to([128, NH, DH])
                with nc.allow_low_precision(reason="attnout bf16 feeds bf16 matmul"):
                    nc.vector.tensor_tensor(
                        ao_all[:, t, :].rearrange("p (h d) -> p h d", h=NH),
                        acc[:].rearrange("p (h d) -> p h d", h=NH),
                        rden_bc, op=OP.mult)

            # prefetch phase C tile-0 inputs (scalar queue is idle by now)
            c_pre = {}
            for pt in (0,):
                a = cstr.tile([128, NIC, 128], F8, tag="hT8_c")
                nc.scalar.dma_start(a[:], hT8_d.ap()[pt])
                b_ = cstr.tile([128, HID], BF16, tag="h_c")
                nc.scalar.dma_start(b_[:], h_d.ap()[pt])
                c_pre[pt] = (a, b_)

        # ===== phase C: memory_out, gate, residual, LayerNorm, output =====
        # gate = h @ Wg_h + attnout @ Wom with Wom = Wo @ Wg_mo folded on the
        # host, so the gate needs only the transposed attnout (atT8) and never
        # waits on memory_out. One-tile software pipeline covers the ScalarE
        # casts: aoT(t) | gate(t-1) | Wo(t).
        with ExitStack() as cctx:
            csb = cctx.enter_context(tc.tile_pool(name="c_sb", bufs=2))
            stp = cctx.enter_context(tc.tile_pool(name="stats", bufs=2))
            tps = cctx.enter_context(tc.tile_pool(name="tp_ps", bufs=2, space="PSUM"))
            mps = cctx.enter_context(tc.tile_pool(name="mo_ps", bufs=1, space="PSUM"))
            gps = cctx.enter_context(tc.tile_pool(name="g_ps", bufs=2, space="PSUM"))

            def emit_gate(hT8_sb, atT_sb):
                g_ps = gps.tile([128, HID], F32, tag="g_ps")
                for icp in range(NIC // 2):
                    for jh in range(NJH):
                        sl = slice(jh * 512, (jh + 1) * 512)
                        nc.tensor.matmul(
                            g_ps[:, sl], hT8_sb[:, 2 * icp:2 * icp + 2, :],
                            wg8_sb[:, 2 * icp:2 * icp + 2, sl],
                            start=(icp == 0), stop=False, perf_mode=DRM)
                for ic in range(NIC):
                    for jh in range(NJH):
                        sl = slice(jh * 512, (jh + 1) * 512)
                        nc.tensor.matmul(
                            g_ps[:, sl], atT_sb[:, ic, :],
                            wom_sb[:, ic, sl],
                            start=False, stop=(ic == NIC - 1))
                return g_ps

            def epilogue(t, g_ps, h_sb, mob_sb, last=False):
                ew = nc.vector if last else nc.gpsimd
                gb_sb = csb.tile([128, HID], F32, tag="gb")
                nc.vector.tensor_add(gb_sb[:], g_ps[:], bgb_sb[:])
                # sigmoid(x) = 0.5*tanh(x/2) + 0.5; 1/WSC descales Wg8/Wom8
                nc.scalar.activation(gb_sb[:], gb_sb[:], AF.Tanh,
                                     scale=0.5 / WSC)
                # aug = h + g*mo = (h + mob) + tanh*mob with mob = 0.5*mo
                u_sb = csb.tile([128, HID], F32, tag="u")
                ew.tensor_add(u_sb[:], h_sb[:], mob_sb[:])
                v_sb = csb.tile([128, HID], F32, tag="v")
                ew.tensor_mul(v_sb[:], gb_sb[:], mob_sb[:])
                nc.vector.scalar_tensor_tensor(
                    u_sb[:], u_sb[:], 0.0, v_sb[:], op0=OP.add, op1=OP.add,
                    accum_out=sum_all[:, t:t + 1])
                # sum of squares: (u+0)*u on DVE, keeping only the accumulator
                nc.vector.scalar_tensor_tensor(
                    v_sb[:], u_sb[:], 0.0, u_sb[:], op0=OP.add, op1=OP.mult,
                    accum_out=ss_all[:, t:t + 1])

                # ---- LayerNorm finalize, per tile, VectorE only ----
                mean = stp.tile([128, 1], F32, tag="mean")
                nc.vector.tensor_scalar_mul(mean[:], sum_all[:, t:t + 1], 1.0 / HID)
                m2 = stp.tile([128, 1], F32, tag="m2")
                nc.vector.tensor_mul(m2[:], mean[:], mean[:])
                nc.vector.tensor_scalar_add(m2[:], m2[:], -LN_EPS)
                vpe = stp.tile([128, 1], F32, tag="vpe")
                nc.vector.scalar_tensor_tensor(
                    vpe[:], ss_all[:, t:t + 1], 1.0 / HID, m2[:],
                    op0=OP.mult, op1=OP.subtract)
                # rstd = 1/sqrt(vpe): quake init + 2 Newton iterations
                y = stp.tile([128, 1], F32, tag="y")
                yi = y[:].bitcast(I32)
                nc.vector.tensor_scalar(
                    yi, vpe[:].bitcast(I32), 1, None,
                    op0=OP.logical_shift_right)
                nc.vector.tensor_scalar(
                    yi, yi, -RSQRT_MAGIC, -1,
                    op0=OP.add, op1=OP.mult)
                yy = stp.tile([128, 1], F32, tag="yy")
                hw = stp.tile([128, 1], F32, tag="hw")
                for _ in range(2):
                    nc.vector.tensor_mul(yy[:], y[:], y[:])
                    nc.vector.tensor_mul(yy[:], yy[:], vpe[:])
                    nc.vector.tensor_scalar(
                        hw[:], yy[:], -0.5, 1.5, op0=OP.mult, op1=OP.add)
                    nc.vector.tensor_mul(y[:], y[:], hw[:])

                # yout = (aug - mean)*rstd*lng + lnb
                nc.vector.scalar_tensor_tensor(
                    u_sb[:], u_sb[:], mean[:], lng_sb[:],
                    op0=OP.subtract, op1=OP.mult)
                yo_sb = csb.tile([128, HID], F32, tag="yo")
                nc.vector.scalar_tensor_tensor(
                    yo_sb[:], u_sb[:], y[:], lnb_sb[:],
                    op0=OP.mult, op1=OP.add)
                nc.sync.dma_start(out_d.ap()[t], yo_sb[:])

            prev = None  # (t, hT8, atT8, h, mob)
            pend = None  # (t, g_ps, h, mob) awaiting epilogue
            for t in range(nt):
                if t in c_pre:
                    hT8_sb, h_sb = c_pre[t]
                else:
                    hT8_sb = cstr.tile([128, NIC, 128], F8, tag="hT8_c")
                    nc.scalar.dma_start(hT8_sb[:], hT8_d.ap()[t])
                    h_sb = cstr.tile([128, HID], BF16, tag="h_c")
                    nc.scalar.dma_start(h_sb[:], h_d.ap()[t])

                # attnout^T via bf16 PE transposes; cast to bf16 (Wo) + f8 (gate)
                at_ps = tps.tile([128, NIC, 128], BF16, tag="tp_ps")
                for ic in range(NIC):
                    nc.tensor.transpose(
                        at_ps[:, ic, :], ao_all[:, t, ic * 128:(ic + 1) * 128],
                        eyeb_sb[:])
                atT_sb = csb.tile([128, NIC, 128], BF16, tag="atT")
                nc.scalar.copy(atT_sb[:], at_ps[:])

                # previous tile's gate fills the PE while atT copies out
                if prev is not None:
                    pt, phT8, patT, ph, pmob = prev
                    pend = (pt, emit_gate(phT8, patT), ph, pmob)

                mo_ps = mps.tile([128, HID], F32, tag="mo_ps")
                for ic in range(NIC):
                    for jh in range(NJH):
                        nc.tensor.matmul(
                            mo_ps[:, jh * 512:(jh + 1) * 512],
                            atT_sb[:, ic, :],
                            wo_sb[:, ic, jh * 512:(jh + 1) * 512],
                            start=(ic == 0), stop=(ic == NIC - 1),
                        )
                # mob holds 0.5*mo (bf16), used only by the aug math
                mob_sb = csb.tile([128, HID], BF16, tag="mob")
                nc.scalar.activation(mob_sb[:], mo_ps[:], AF.Copy, scale=0.5)

                if pend is not None:
                    epilogue(*pend)
                    pend = None
                prev = (t, hT8_sb, atT_sb, h_sb, mob_sb)

            pt, phT8, patT, ph, pmob = prev
            g_last = emit_gate(phT8, patT)
            epilogue(pt, g_last, ph, pmob, last=True)

        cstr_cm.__exit__(None, None, None)
        pAO_cm.__exit__(None, None, None)   # release attnout
        pWO_cm.__exit__(None, None, None)   # release Wo/Wg

    nc.compile()
    return nc


def _prep_core(hs, mk, mv, nt, bf, e4):
    """Host-side layout prep for one core's shard (transpose/reshape + casts)."""
    hT = np.ascontiguousarray(
        hs.reshape(nt, 128, NIC, 128).transpose(0, 3, 2, 1))      # [t,p,ic,b]
    h = np.ascontiguousarray(hs.reshape(nt, 128, HID)).astype(bf)
    mkT = np.ascontiguousarray(
        mk.reshape(nt, 128, TOPK, NIC, 128).transpose(0, 2, 4, 3, 1))
    mvT = np.ascontiguousarray(
        mv.reshape(nt, 128, TOPK, NIC, 128).transpose(0, 2, 4, 3, 1))
    return hT.astype(bf), hT.astype(e4), h, mkT.astype(e4), mvT.astype(bf)


def kernel(**inputs):
    hs = np.asarray(inputs["hidden_state"], dtype=np.float32)
    mk = np.asarray(inputs["memory_keys"], dtype=np.float32)
    mv = np.asarray(inputs["memory_values"], dtype=np.float32)

    import ml_dtypes
    bf = ml_dtypes.bfloat16
    e4 = ml_dtypes.float8_e4m3
    wq = np.ascontiguousarray(
        np.asarray(inputs["Wq"], np.float32).reshape(NIC, 128, HID).transpose(1, 0, 2)).astype(bf)
    wk8 = np.ascontiguousarray(
        (np.asarray(inputs["Wk"], np.float32) * WSC).reshape(NIC, 128, HID).transpose(1, 0, 2)).astype(e4)
    wv = np.ascontiguousarray(
        np.asarray(inputs["Wv"], np.float32).reshape(NIC, 128, HID).transpose(1, 0, 2)).astype(bf)
    wo = np.ascontiguousarray(
        np.asarray(inputs["Wo"], np.float32).reshape(NIC, 128, HID).transpose(1, 0, 2)).astype(bf)
    wg_f = np.asarray(inputs["Wg"], np.float32)
    wg8 = np.ascontiguousarray(
        (wg_f[:HID] * WSC).reshape(NIC, 128, HID).transpose(1, 0, 2)).astype(e4)
    # gate mo-half folded through Wo on the host: ao @ (Wo @ Wg_mo)
    wom_f = (np.asarray(inputs["Wo"], np.float32) @ wg_f[HID:]) * WSC
    wom = np.ascontiguousarray(
        wom_f.reshape(NIC, 128, HID).transpose(1, 0, 2)).astype(bf)
    bgb = np.ascontiguousarray(
        np.broadcast_to(np.asarray(inputs["bg"], np.float32), (128, HID))).astype(bf)
    lng = np.ascontiguousarray(
        np.broadcast_to(np.asarray(inputs["ln_g"], np.float32), (128, HID))).astype(bf)
    lnb = np.ascontiguousarray(
        np.broadcast_to(np.asarray(inputs["ln_b"], np.float32), (128, HID))).astype(bf)
    eyeb = np.eye(128, dtype=np.float32).astype(bf)

    if "nc" not in _CACHE:
        _CACHE["nc"] = _build(NT)
    nc = _CACHE["nc"]

    in_maps = []
    for c in range(N_CORES):
        sl = slice(c * BC, (c + 1) * BC)
        hTb, hT8, h, mkT8, mvT = _prep_core(hs[sl], mk[sl], mv[sl], NT, bf, e4)
        in_maps.append({
            "hTb": hTb, "hT8": hT8, "h": h,
            "mkT8": mkT8, "mvT": mvT,
            "Wq": wq, "Wk8": wk8, "Wv": wv, "Wo": wo, "Wg8": wg8,
            "Wom": wom,
            "bgB": bgb, "eyeb": eyeb, "lngB": lng, "lnbB": lnb,
        })

    res = run_bass_kernel_spmd(nc, in_maps, core_ids=list(range(N_CORES)),
                               trace=TRACE)
    kernel.last_result = res
    out = np.concatenate(
        [r["out"].reshape(BC, HID) for r in res.results], axis=0)
    return out


kernel.last_result = None


# revision 30
# speedup vs baseline: 1.1790x; 1.0078x over previous
"""Trainium2 Bass kernel for nn_EngramModule: single-query top-k memory attention
with gated residual + LayerNorm, data-parallel across 8 NeuronCores.

Contract: kernel(**inputs) takes the FULL unsharded inputs and returns the FULL
(8192, 1024) float32 output.

Per-core pipeline (1024 batch rows, 8 row-tiles of 128):
  A+B (fused): per tile, Q = h @ Wq (bf16) then per k-slot: Kp projection in
      fp8e4 DoubleRow (2 contraction chunks per instruction, 2x bf16 rate;
      Wk host-scaled by 32, folded into the exp scale); Vp projection in
      bf16; scores = per-head reduce of q*Kp (DVE); e = exp(scores*scale)
      per-k on ScalarE; weighted V: mult on DVE, running add on GpSimd.
  C:  software-pipelined by one tile on the PE
      (aoT(t) | gate2(t-1) | Wo(t) | gate1(t) | moT(t)) so the ScalarE
      PSUM->SBUF casts always have a PE block in front of their consumer.
      memory_out = attnout @ Wo (bf16); gate = [h|mo] @ Wg in fp8e4
      DoubleRow (Wg host-scaled by 32, mo-half by another 2 since the
      kernel feeds 0.5*mo; sigmoid(x) = 0.5*tanh(x/2)+0.5); aug = h+g*mo;
      LayerNorm per tile: sums via DVE accumulators, Newton rsqrt.

fp8 is used only where the quantization error budget allows (K-side + gate,
~0.015 rel err vs the 2e-2 gate); V/Q/Wo stay bf16. Bulk weights and phase C
inputs ride the ScalarE HWDGE queue; per-(tile,k) activation streams and
outputs ride the SyncE queue. Activations are pre-laid-out on host (pure
transpose/reshape + dtype casts) so contraction dims sit on SBUF partitions.
"""

import os
import sys

import numpy as np

for _p in ("/opt/trn_rl_repo", "/root/.axon_site/_ro/trn_rl_repo"):
    if os.path.isdir(_p) and _p not in sys.path:
        sys.path.insert(0, _p)

from contextlib import ExitStack

import concourse.bacc as bacc
import concourse.mybir as mybir
import concourse.tile as tile
from concourse.bass_utils import run_bass_kernel_spmd

F32 = mybir.dt.float32
BF16 = mybir.dt.bfloat16
F8 = mybir.dt.float8e4
I32 = mybir.dt.int32
AX = mybir.AxisListType
OP = mybir.AluOpType
AF = mybir.ActivationFunctionType
DRM = mybir.MatmulPerfMode.DoubleRow

N_CORES = 8
B = 8192
HID = 1024
NH = 16
DH = 64
TOPK = 8
LN_EPS = 1e-5

BC = B // N_CORES          # rows per core = 1024
NT = BC // 128             # row-tiles per core = 8
NIC = HID // 128           # 128-row contraction chunks = 8
NJH = HID // 512           # 512-wide output halves = 2
WSC = 32.0                 # host scale on fp8 weights
SCALE = DH ** -0.5
RSQRT_MAGIC = 0x5F3759DF

# Set by test.py to collect a profile; grading path leaves this off.
TRACE = False

_CACHE = {}


def _build(nt=NT):
    nc = bacc.Bacc("TRN2", target_bir_lowering=False, debug=False,
                   num_devices=N_CORES)

    # ---- DRAM parameters (per-core shard, host-prepped layouts) ----
    h_d = nc.declare_dram_parameter("h", [nt, 128, HID], BF16, isOutput=False)
    hTb_d = nc.declare_dram_parameter("hTb", [nt, 128, NIC, 128], BF16, isOutput=False)
    hT8_d = nc.declare_dram_parameter("hT8", [nt, 128, NIC, 128], F8, isOutput=False)
    mkT8_d = nc.declare_dram_parameter("mkT8", [nt, TOPK, 128, NIC, 128], F8, isOutput=False)
    mvT_d = nc.declare_dram_parameter("mvT", [nt, TOPK, 128, NIC, 128], BF16, isOutput=False)
    wq_d = nc.declare_dram_parameter("Wq", [128, NIC, HID], BF16, isOutput=False)
    wk8_d = nc.declare_dram_parameter("Wk8", [128, NIC, HID], F8, isOutput=False)
    wv_d = nc.declare_dram_parameter("Wv", [128, NIC, HID], BF16, isOutput=False)
    wo_d = nc.declare_dram_parameter("Wo", [128, NIC, HID], BF16, isOutput=False)
    wg8_d = nc.declare_dram_parameter("Wg8", [128, NIC, HID], F8, isOutput=False)
    wom_d = nc.declare_dram_parameter("Wom", [128, NIC, HID], BF16, isOutput=False)
    bgb_d = nc.declare_dram_parameter("bgB", [128, HID], BF16, isOutput=False)
    eyeb_d = nc.declare_dram_parameter("eyeb", [128, 128], BF16, isOutput=False)
    ones_d = nc.declare_dram_parameter("ones", [128, 128], BF16, isOutput=False)
    lng_d = nc.declare_dram_parameter("lngB", [128, HID], BF16, isOutput=False)
    lnb_d = nc.declare_dram_parameter("lnbB", [128, HID], BF16, isOutput=False)
    out_d = nc.declare_dram_parameter("out", [nt, 128, HID], F32, isOutput=True)

    def load_w(tile_sb, dram, nchunk):
        # bulk weights on the ScalarE HWDGE queue, chunked so the first
        # dependent matmul only waits for its own chunk
        for ic in range(nchunk):
            nc.scalar.dma_start(tile_sb[:, ic, :], dram.ap()[:, ic, :])

    with ExitStack() as octx:
        tc = octx.enter_context(tile.TileContext(nc))

        pers = octx.enter_context(tc.tile_pool(name="pers", bufs=1))
        sum_all = pers.tile([128, nt], F32, tag="sum_all")
        ss_all = pers.tile([128, nt], F32, tag="ss_all")
        # phase C constants, DMAed during the A+B head so C never waits
        eyeb_sb = pers.tile([128, 128], BF16, tag="eyeb")
        ones_sb = pers.tile([128, 128], BF16, tag="ones")
        bgb_sb = pers.tile([128, HID], BF16, tag="bgb")
        lng_sb = pers.tile([128, HID], BF16, tag="lng")
        lnb_sb = pers.tile([128, HID], BF16, tag="lnb")

        # Wo/Wg are loaded during A+B (scalar queue) and consumed in C
        pWO_cm = tc.tile_pool(name="pWO", bufs=1); pWO = pWO_cm.__enter__()
        wo_sb = pWO.tile([128, NIC, HID], BF16, tag="wo")
        wg8_sb = pWO.tile([128, NIC, HID], F8, tag="wg8")
        wom_sb = pWO.tile([128, NIC, HID], BF16, tag="wom")

        # attnout stays SBUF-resident from B into C (bf16, feeds transposes)
        pAO_cm = tc.tile_pool(name="pAO", bufs=1); pAO = pAO_cm.__enter__()
        ao_all = pAO.tile([128, nt, HID], BF16, tag="ao_all")

        # phase C per-tile input stream (created here so B can prefetch t=0)
        cstr_cm = tc.tile_pool(name="c_str", bufs=3); cstr = cstr_cm.__enter__()

        # ========== phase A+B: Q projection fused into attention ==========
        with ExitStack() as bctx:
            wqp = bctx.enter_context(tc.tile_pool(name="wq", bufs=1))
            hp = bctx.enter_context(tc.tile_pool(name="hT_a", bufs=3))
            qp = bctx.enter_context(tc.tile_pool(name="qq", bufs=2))
            wkv = bctx.enter_context(tc.tile_pool(name="wkv", bufs=1))
            mp = bctx.enter_context(tc.tile_pool(name="mkv", bufs=4))
            kvps = bctx.enter_context(tc.tile_pool(name="kv_ps", bufs=2, space="PSUM"))
            sp = bctx.enter_context(tc.tile_pool(name="scr", bufs=2))
            accp = bctx.enter_context(tc.tile_pool(name="acc", bufs=2))
            ep = bctx.enter_context(tc.tile_pool(name="e", bufs=2))

            wq_sb = wqp.tile([128, NIC, HID], BF16, tag="wq")
            wk8_sb = wkv.tile([128, NIC, HID], F8, tag="wk8")
            wv_sb = wkv.tile([128, NIC, HID], BF16, tag="wv")
            load_w(wq_sb, wq_d, NIC)
            load_w(wk8_sb, wk8_d, NIC)
            load_w(wv_sb, wv_d, NIC)
            hT_pre = {}
            for pt in (0, 1):
                ht = hp.tile([128, NIC, 128], BF16, tag="hT")
                nc.sync.dma_start(ht[:], hTb_d.ap()[pt])
                hT_pre[pt] = ht
            preload = {}
            for (pt, pk) in ((0, 0),):
                a = mp.tile([128, NIC, 128], F8, tag="mkT8")
                nc.sync.dma_start(a[:], mkT8_d.ap()[pt, pk])
                b_ = mp.tile([128, NIC, 128], BF16, tag="mvT")
                nc.sync.dma_start(b_[:], mvT_d.ap()[pt, pk])
                preload[(pt, pk)] = (a, b_)

            for t in range(nt):
                # phase C weights/constants trickle in on the sync queue
                # (its own sequencer, no ScalarE cost) spread across tiles
                if t == 1:
                    for ic in range(NIC):
                        nc.sync.dma_start(wo_sb[:, ic, :], wo_d.ap()[:, ic, :])
                elif t == 2:
                    for ic in range(NIC):
                        nc.sync.dma_start(wg8_sb[:, ic, :], wg8_d.ap()[:, ic, :])
                elif t == 3:
                    for ic in range(NIC):
                        nc.sync.dma_start(wom_sb[:, ic, :], wom_d.ap()[:, ic, :])
                    nc.sync.dma_start(eyeb_sb[:], eyeb_d.ap())
                    nc.sync.dma_start(ones_sb[:], ones_d.ap())
                    nc.sync.dma_start(bgb_sb[:], bgb_d.ap())
                    nc.sync.dma_start(lng_sb[:], lng_d.ap())
                    nc.sync.dma_start(lnb_sb[:], lnb_d.ap())
                if t in hT_pre:
                    hT_t = hT_pre[t]
                else:
                    hT_t = hp.tile([128, NIC, 128], BF16, tag="hT")
                    nc.sync.dma_start(hT_t[:], hTb_d.ap()[t])

                # Q projection for this tile (PSUM slot shared with kp)
                q_ps = kvps.tile([128, HID], F32, tag="kp")
                for ic in range(NIC):
                    for jh in range(NJH):
                        nc.tensor.matmul(
                            q_ps[:, jh * 512:(jh + 1) * 512],
                            hT_t[:, ic, :],
                            wq_sb[:, ic, jh * 512:(jh + 1) * 512],
                            start=(ic == 0), stop=(ic == NIC - 1),
                        )
                q_t = qp.tile([128, HID], BF16, tag="q")
                nc.scalar.copy(q_t[:], q_ps[:])

                acc = accp.tile([128, HID], F32, tag="acc")
                e_all = ep.tile([128, TOPK, NH], F32, tag="e_all")
                for k in range(TOPK):
                    if (t, k) in preload:
                        mkT8, mvT = preload[(t, k)]
                    else:
                        mkT8 = mp.tile([128, NIC, 128], F8, tag="mkT8")
                        nc.sync.dma_start(mkT8[:], mkT8_d.ap()[t, k])
                        mvT = mp.tile([128, NIC, 128], BF16, tag="mvT")
                        nc.sync.dma_start(mvT[:], mvT_d.ap()[t, k])

                    # Kp in fp8 DoubleRow: 2 contraction chunks per matmul
                    kp_ps = kvps.tile([128, HID], F32, tag="kp")
                    for icp in range(NIC // 2):
                        for jh in range(NJH):
                            nc.tensor.matmul(
                                kp_ps[:, jh * 512:(jh + 1) * 512],
                                mkT8[:, 2 * icp:2 * icp + 2, :],
                                wk8_sb[:, 2 * icp:2 * icp + 2, jh * 512:(jh + 1) * 512],
                                start=(icp == 0), stop=(icp == NIC // 2 - 1),
                                perf_mode=DRM,
                            )
                    # Vp in bf16
                    vp_ps = kvps.tile([128, HID], F32, tag="vp")
                    for ic in range(NIC):
                        for jh in range(NJH):
                            nc.tensor.matmul(
                                vp_ps[:, jh * 512:(jh + 1) * 512],
                                mvT[:, ic, :],
                                wv_sb[:, ic, jh * 512:(jh + 1) * 512],
                                start=(ic == 0), stop=(ic == NIC - 1),
                            )

                    # scores for all 16 heads of this k-slot
                    p_scr = sp.tile([128, HID], BF16, tag="p")
                    nc.vector.tensor_mul(p_scr[:], q_t[:], kp_ps[:])
                    s_k = ep.tile([128, NH], F32, tag="s_k")
                    nc.vector.reduce_sum(
                        s_k[:], p_scr[:].rearrange("p (h d) -> p h d", h=NH), axis=AX.X)
                    # e = exp(scores * DH**-0.5 / WSC); logits ~N(0,1), no max-sub
                    nc.scalar.activation(e_all[:, k, :], s_k[:], AF.Exp,
                                         scale=SCALE / WSC)

                    # weighted V accumulate: DVE mult, GpSimd running add
                    e_bc = e_all[:, k, :].unsqueeze(2).broadcast_to([128, NH, DH])
                    dst = acc if k == 0 else sp.tile([128, HID], F32, tag="pv")
                    nc.vector.tensor_tensor(
                        dst[:].rearrange("p (h d) -> p h d", h=NH),
                        vp_ps[:].rearrange("p (h d) -> p h d", h=NH),
                        e_bc, op=OP.mult)
                    if k > 0:
                        nc.gpsimd.tensor_add(acc[:], acc[:], dst[:])

                # normalize: attnout = acc * (1/sum_k e), written bf16
                den = ep.tile([128, NH], F32, tag="den")
                nc.vector.reduce_sum(
                    den[:], e_all[:].rearrange("p k h -> p h k"), axis=AX.X)
                rden = ep.tile([128, NH], F32, tag="rden")
                nc.vector.reciprocal(rden[:], den[:])
                rden_bc = rden[:].unsqueeze(2).broadcast_to([128, NH, DH])
                with nc.allow_low_precision(reason="attnout bf16 feeds bf16 matmul"):
                    nc.vector.tensor_tensor(
                        ao_all[:, t, :].rearrange("p (h d) -> p h d", h=NH),
                        acc[:].rearrange("p (h d) -> p h d", h=NH),
                        rden_bc, op=OP.mult)

            # prefetch phase C tile-0 inputs (scalar queue is idle by now)
            c_pre = {}
            for pt in (0,):
                a = cstr.tile([128, NIC, 128], F8, tag="hT8_c")
                nc.scalar.dma_start(a[:], hT8_d.ap()[pt])
                b_ = cstr.tile([128, HID], BF16, tag="h_c")
                nc.scalar.dma_start(b_[:], h_d.ap()[pt])
                c_pre[pt] = (a, b_)

        # ===== phase C: memory_out, gate, residual, LayerNorm, output =====
        # gate = h @ Wg_h + attnout @ Wom with Wom = Wo @ Wg_mo folded on the
        # host, so the gate needs only the transposed attnout (atT8) and never
        # waits on memory_out. One-tile software pipeline covers the ScalarE
        # casts: aoT(t) | gate(t-1) | Wo(t).
        with ExitStack() as cctx:
            csb = cctx.enter_context(tc.tile_pool(name="c_sb", bufs=2))
            stp = cctx.enter_context(tc.tile_pool(name="stats", bufs=2))
            tps = cctx.enter_context(tc.tile_pool(name="tp_ps", bufs=2, space="PSUM"))
            mps = cctx.enter_context(tc.tile_pool(name="mo_ps", bufs=1, space="PSUM"))
            gps = cctx.enter_context(tc.tile_pool(name="g_ps", bufs=2, space="PSUM"))

            def emit_gate(hT8_sb, atT_sb):
                g_ps = gps.tile([128, HID], F32, tag="g_ps")
                for icp in range(NIC // 2):
                    for jh in range(NJH):
                        sl = slice(jh * 512, (jh + 1) * 512)
                        nc.tensor.matmul(
                            g_ps[:, sl], hT8_sb[:, 2 * icp:2 * icp + 2, :],
                            wg8_sb[:, 2 * icp:2 * icp + 2, sl],
                            start=(icp == 0), stop=False, perf_mode=DRM)
                for ic in range(NIC):
                    for jh in range(NJH):
                        sl = slice(jh * 512, (jh + 1) * 512)
                        nc.tensor.matmul(
                            g_ps[:, sl], atT_sb[:, ic, :],
                            wom_sb[:, ic, sl],
                            start=False, stop=False)
                # + bg: ones.T @ bgb_row adds 32*bg[j] to every row
                for jh in range(NJH):
                    sl = slice(jh * 512, (jh + 1) * 512)
                    nc.tensor.matmul(
                        g_ps[:, sl], ones_sb[:], bgb_sb[:, sl],
                        start=False, stop=(jh == NJH - 1))
                return g_ps

            def epilogue(t, g_ps, h_sb, mob_sb, last=False):
                ew = nc.vector if last else nc.gpsimd
                # sigmoid(x) = 0.5*tanh(x/2) + 0.5; 1/WSC descales Wg8/Wom;
                # bg already folded into g_ps by the gate's ones-matmul
                gb_sb = csb.tile([128, HID], F32, tag="gb")
                nc.scalar.activation(gb_sb[:], g_ps[:], AF.Tanh,
                                     scale=0.5 / WSC)
                # aug = h + g*mo = (h + mob) + tanh*mob with mob = 0.5*mo
                u_sb = csb.tile([128, HID], F32, tag="u")
                ew.tensor_add(u_sb[:], h_sb[:], mob_sb[:])
                v_sb = csb.tile([128, HID], F32, tag="v")
                ew.tensor_mul(v_sb[:], gb_sb[:], mob_sb[:])
                nc.vector.scalar_tensor_tensor(
                    u_sb[:], u_sb[:], 0.0, v_sb[:], op0=OP.add, op1=OP.add,
                    accum_out=sum_all[:, t:t + 1])
                # square's tensor output is scrap; we only keep the accumulator
                nc.scalar.activation(
                    v_sb[:], u_sb[:], AF.Square, accum_out=ss_all[:, t:t + 1])

                # ---- LayerNorm finalize, per tile, VectorE only ----
                mean = stp.tile([128, 1], F32, tag="mean")
                nc.vector.tensor_scalar_mul(mean[:], sum_all[:, t:t + 1], 1.0 / HID)
                m2 = stp.tile([128, 1], F32, tag="m2")
                nc.vector.tensor_mul(m2[:], mean[:], mean[:])
                nc.vector.tensor_scalar_add(m2[:], m2[:], -LN_EPS)
                vpe = stp.tile([128, 1], F32, tag="vpe")
                nc.vector.scalar_tensor_tensor(
                    vpe[:], ss_all[:, t:t + 1], 1.0 / HID, m2[:],
                    op0=OP.mult, op1=OP.subtract)
                # rstd = 1/sqrt(vpe): quake init + 2 Newton iterations
                y = stp.tile([128, 1], F32, tag="y")
                yi = y[:].bitcast(I32)
                nc.vector.tensor_scalar(
                    yi, vpe[:].bitcast(I32), 1, None,
                    op0=OP.logical_shift_right)
                nc.vector.tensor_scalar(
                    yi, yi, -RSQRT_MAGIC, -1,
                    op0=OP.add, op1=OP.mult)
                yy = stp.tile([128, 1], F32, tag="yy")
                hw = stp.tile([128, 1], F32, tag="hw")
                for _ in range(2):
                    nc.vector.tensor_mul(yy[:], y[:], y[:])
                    nc.vector.tensor_mul(yy[:], yy[:], vpe[:])
                    nc.vector.tensor_scalar(
                        hw[:], yy[:], -0.5, 1.5, op0=OP.mult, op1=OP.add)
                    nc.vector.tensor_mul(y[:], y[:], hw[:])

                # yout = (aug - mean)*rstd*lng + lnb
                nc.vector.scalar_tensor_tensor(
                    u_sb[:], u_sb[:], mean[:], lng_sb[:],
                    op0=OP.subtract, op1=OP.mult)
                yo_sb = csb.tile([128, HID], F32, tag="yo")
                nc.vector.scalar_tensor_tensor(
                    yo_sb[:], u_sb[:], y[:], lnb_sb[:],
                    op0=OP.mult, op1=OP.add)
                nc.sync.dma_start(out_d.ap()[t], yo_sb[:])

            prev = None  # (t, hT8, atT8, h, mob)
            pend = None  # (t, g_ps, h, mob) awaiting epilogue
            for t in range(nt):
                if t in c_pre:
                    hT8_sb, h_sb = c_pre[t]
                else:
                    hT8_sb = cstr.tile([128, NIC, 128], F8, tag="hT8_c")
                    nc.scalar.dma_start(hT8_sb[:], hT8_d.ap()[t])
                    h_sb = cstr.tile([128, HID], BF16, tag="h_c")
                    nc.scalar.dma_start(h_sb[:], h_d.ap()[t])

                # attnout^T via bf16 PE transposes; cast to bf16 (Wo) + f8 (gate)
                at_ps = tps.tile([128, NIC, 128], BF16, tag="tp_ps")
                for ic in range(NIC):
                    nc.tensor.transpose(
                        at_ps[:, ic, :], ao_all[:, t, ic * 128:(ic + 1) * 128],
                        eyeb_sb[:])
                atT_sb = csb.tile([128, NIC, 128], BF16, tag="atT")
                nc.scalar.copy(atT_sb[:], at_ps[:])

                # previous tile's gate fills the PE while atT copies out
                if prev is not None:
                    pt, phT8, patT, ph, pmob = prev
                    pend = (pt, emit_gate(phT8, patT), ph, pmob)
                g_last = emit_gate(hT8_sb, atT_sb) if t == nt - 1 else None

                mo_ps = mps.tile([128, HID], F32, tag="mo_ps")
                for ic in range(NIC):
                    for jh in range(NJH):
                        nc.tensor.matmul(
                            mo_ps[:, jh * 512:(jh + 1) * 512],
                            atT_sb[:, ic, :],
                            wo_sb[:, ic, jh * 512:(jh + 1) * 512],
                            start=(ic == 0), stop=(ic == NIC - 1),
                        )
                # mob holds 0.5*mo (bf16), used only by the aug math
                mob_sb = csb.tile([128, HID], BF16, tag="mob")
                nc.scalar.activation(mob_sb[:], mo_ps[:], AF.Copy, scale=0.5)

                if pend is not None:
                    epilogue(*pend)
                    pend = None
                prev = (t, hT8_sb, atT_sb, h_sb, mob_sb)

            pt, phT8, patT, ph, pmob = prev
            epilogue(pt, g_last, ph, pmob, last=True)

        cstr_cm.__exit__(None, None, None)
        pAO_cm.__exit__(None, None, None)   # release attnout
        pWO_cm.__exit__(None, None, None)   # release Wo/Wg

    nc.compile()
    return nc


def _prep_core(hs, mk, mv, nt, bf, e4):
    """Host-side layout prep for one core's shard (transpose/reshape + casts)."""
    hT = np.ascontiguousarray(
        hs.reshape(nt, 128, NIC, 128).transpose(0, 3, 2, 1))      # [t,p,ic,b]
    h = np.ascontiguousarray(hs.reshape(nt, 128, HID)).astype(bf)
    mkT = np.ascontiguousarray(
        mk.reshape(nt, 128, TOPK, NIC, 128).transpose(0, 2, 4, 3, 1))
    mvT = np.ascontiguousarray(
        mv.reshape(nt, 128, TOPK, NIC, 128).transpose(0, 2, 4, 3, 1))
    return hT.astype(bf), hT.astype(e4), h, mkT.astype(e4), mvT.astype(bf)


def kernel(**inputs):
    hs = np.asarray(inputs["hidden_state"], dtype=np.float32)
    mk = np.asarray(inputs["memory_keys"], dtype=np.float32)
    mv = np.asarray(inputs["memory_values"], dtype=np.float32)

    import ml_dtypes
    bf = ml_dtypes.bfloat16
    e4 = ml_dtypes.float8_e4m3
    wq = np.ascontiguousarray(
        np.asarray(inputs["Wq"], np.float32).reshape(NIC, 128, HID).transpose(1, 0, 2)).astype(bf)
    wk8 = np.ascontiguousarray(
        (np.asarray(inputs["Wk"], np.float32) * WSC).reshape(NIC, 128, HID).transpose(1, 0, 2)).astype(e4)
    wv = np.ascontiguousarray(
        np.asarray(inputs["Wv"], np.float32).reshape(NIC, 128, HID).transpose(1, 0, 2)).astype(bf)
    wo = np.ascontiguousarray(
        np.asarray(inputs["Wo"], np.float32).reshape(NIC, 128, HID).transpose(1, 0, 2)).astype(bf)
    wg_f = np.asarray(inputs["Wg"], np.float32)
    wg8 = np.ascontiguousarray(
        (wg_f[:HID] * WSC).reshape(NIC, 128, HID).transpose(1, 0, 2)).astype(e4)
    # gate mo-half folded through Wo on the host: ao @ (Wo @ Wg_mo)
    wom_f = (np.asarray(inputs["Wo"], np.float32) @ wg_f[HID:]) * WSC
    wom = np.ascontiguousarray(
        wom_f.reshape(NIC, 128, HID).transpose(1, 0, 2)).astype(bf)
    bgb = np.ascontiguousarray(np.broadcast_to(
        np.asarray(inputs["bg"], np.float32) * (WSC / 128.0), (128, HID))).astype(bf)
    ones = np.ones((128, 128), dtype=np.float32).astype(bf)
    lng = np.ascontiguousarray(
        np.broadcast_to(np.asarray(inputs["ln_g"], np.float32), (128, HID))).astype(bf)
    lnb = np.ascontiguousarray(
        np.broadcast_to(np.asarray(inputs["ln_b"], np.float32), (128, HID))).astype(bf)
    eyeb = np.eye(128, dtype=np.float32).astype(bf)

    if "nc" not in _CACHE:
        _CACHE["nc"] = _build(NT)
    nc = _CACHE["nc"]

    in_maps = []
    for c in range(N_CORES):
        sl = slice(c * BC, (c + 1) * BC)
        hTb, hT8, h, mkT8, mvT = _prep_core(hs[sl], mk[sl], mv[sl], NT, bf, e4)
        in_maps.append({
            "hTb": hTb, "hT8": hT8, "h": h,
            "mkT8": mkT8, "mvT": mvT,
            "Wq": wq, "Wk8": wk8, "Wv": wv, "Wo": wo, "Wg8": wg8,
            "Wom": wom,
            "bgB": bgb, "eyeb": eyeb, "ones": ones, "lngB": lng, "lnbB": lnb,
        })

    res = run_bass_kernel_spmd(nc, in_maps, core_ids=list(range(N_CORES)),
                               trace=TRACE)
    kernel.last_result = res
    out = np.concatenate(
        [r["out"].reshape(BC, HID) for r in res.results], axis=0)
    return out


kernel.last_result = None
